# revision 42
# baseline (speedup 1.0000x reference)
"""PointMatcher kernel for Trainium2 (8 NeuronCores, SPMD).

Sharding: pred_points (N=1024) split across 8 cores (128 rows = SBUF
partitions per core); gt_points replicated.

Per p-coordinate (P=20) the squared point-pair distance is a rank-4
bilinear form:
    s_p[n,m] = |p_n|^2 + |g_m|^2 - 2 p_n.g_m
             = [px, py, pn2, 1] . [-2gx, -2gy, 1, gn2]     (K=4)
The augmented operands are built host-side (cheap O(N*P) layout work) and
DMA'd in with each p-group at a 32-aligned partition base (PE quadrant
constraint). PE computes s via K=4 matmuls (1 col/cycle regardless of K),
ACT does sqrt(s/400 + eps) straight out of PSUM (eps guards fp32
cancellation-driven negatives; /400 folds the mean over P=20), DVE reduces
over P and does min/argmin per m-block, GPSIMD indirect-DMA gathers the
matched gt rows.
"""

import numpy as np

import concourse.bass as bass
import concourse.tile as tile
from concourse import bacc, mybir
from concourse.bass_utils import run_bass_kernel_spmd
from concourse.tile import add_dep_helper

N_CORES = 8
N, P, C = 1024, 20, 2
M = 2048
NL = N // N_CORES            # 128 pred rows per core == SBUF partitions
PC = P * C                   # 40
MB = 512                     # m-block (one PSUM bank of f32)
NJ = M // MB                 # 4 m-blocks
QUAD = 4                     # p's per PSUM tile / ACT instruction
NQ = P // QUAD               # 5
TRIO = 3                     # p-groups per partition layer (K base in {0,32,64})
SLOTS = 7                    # ceil(P / TRIO) free slots
P_PAD = TRIO * SLOTS         # 21 p-groups after zero padding
F32 = mybir.dt.float32
F16 = mybir.dt.float16
I32 = mybir.dt.int32
FSQRT = mybir.ActivationFunctionType.Sqrt
FEXP = mybir.ActivationFunctionType.Exp
OP = mybir.AluOpType
AX = mybir.AxisListType

SCALE = 1.0 / 400.0          # sqrt(s/400) == sqrt(s)/20 -> mean over P folded in
BIAS = 1e-5 * SCALE          # clamp fp16-split cancellation (s >= -1.8e-6 observed)
BIG = 4096.0                 # > any valid m index
DIST_THRESHOLD = 2.0
# m-blocks: small edge blocks shorten the DVE lead-in and tail
BLOCKS = [(0, 128), (128, 256), (384, 512), (896, 512), (1408, 512), (1920, 128)]
# exp(-x) on [0, 2.2], degree-7 (2.3e-6 max rel err in fp32); past the
# threshold the mask zeroes conf, so accuracy there is irrelevant
EXP_COEF = [
    0.999998982993151, -0.9999732918541867, 0.4998190768987987,
    -0.1661297557924529, 0.040819438656213146, -0.007559750384418043,
    0.0009681929279338809, -6.342769096779997e-05,
]

_CACHE: dict = {}


def _build():
    nc = bacc.Bacc("TRN2", target_bir_lowering=False, debug=False)

    pred_d = nc.dram_tensor("pred_aug", [12 * P_PAD, NL], F16, kind="ExternalInput")
    gta_d = nc.dram_tensor("gt_aug", [12 * P_PAD, M], F16, kind="ExternalInput")
    gt_d = nc.dram_tensor("gt", [M, P, C], F32, kind="ExternalInput")
    matched_d = nc.dram_tensor("matched", [NL, P, C], F32, kind="ExternalOutput")
    conf_d = nc.dram_tensor("conf", [NL, 1], F32, kind="ExternalOutput")
    idx_d = nc.dram_tensor("idx", [NL, 1], I32, kind="ExternalOutput")

    with tile.TileContext(nc) as tc:
        with (
            tc.tile_pool(name="sb", bufs=1) as sb,
            tc.tile_pool(name="dbuf", bufs=3) as dbuf,
            tc.tile_pool(name="blk", bufs=2) as blk,
            tc.tile_pool(name="ps", bufs=2, space=bass.MemorySpace.PSUM) as ps,
        ):
            # ---- load operands: p-group (3t+e) lives at partitions
            # [32e, 32e+12), free slot t. ----
            # One K=12 matmul per (p, block) computes the exact-enough product:
            # s = [A_hi, A_hi, A_lo] . [B_hi, B_lo, B_hi]  (A_lo.B_lo ~2^-24 dropped)
            # gt tensor split into head (slots 0-1) / tail (slots 2-6) tiles
            # so the first matmuls start after a small fraction of the DMA.
            predT = sb.tile([128, SLOTS, NL], F16, tag="predT")
            gtTa = sb.tile([128, 2, M], F16, tag="gtTa")
            gtTb = sb.tile([128, SLOTS - 2, M], F16, tag="gtTb")
            pred_v = pred_d.ap().rearrange(
                "(t e c) n -> e c t n", t=SLOTS, e=TRIO, c=12
            )
            gta_v = gta_d.ap().rearrange(
                "(t e c) m -> e c t m", t=SLOTS, e=TRIO, c=12
            )
            for e in range(TRIO):
                nc.sync.dma_start(
                    gtTa[32 * e:32 * e + 12, :, :], gta_v[e][:, 0:2, :]
                )
                nc.sync.dma_start(
                    predT[32 * e:32 * e + 12, :, :], pred_v[e]
                )
            for e in range(TRIO):
                nc.sync.dma_start(
                    gtTb[32 * e:32 * e + 12, :, :], gta_v[e][:, 2:, :]
                )

            iota_t = sb.tile([NL, M], F32, tag="iota")
            nc.gpsimd.iota(
                iota_t[:],
                [[1, M]],
                channel_multiplier=0,
                allow_small_or_imprecise_dtypes=True,
            )
            bias_t = sb.tile([NL, 1], F32, tag="bias")
            nc.gpsimd.memset(bias_t[:], BIAS)
            # data-independent dummy sqrt: pulls the ACT table load to t=0
            scratch = sb.tile([NL, 1], F32, tag="scr")
            nc.scalar.activation(scratch[:], bias_t[:], FSQRT)

            # ---- main loop: matmul -> sqrt -> reduce over P -> block min/argmin ----
            NB = len(BLOCKS)
            acc = sb.tile([NL, M], F32, tag="acc")
            mnp = sb.tile([NL, NB], F32, tag="mnp")
            amins = sb.tile([NL, NB], F32, tag="amins")

            def emit_chain(j, m0, mb, on_pool):
                # min/argmin candidate chain for block j; mask/cand on the
                # pool mid-loop (its inputs are ready well before the pool
                # reaches them), on DVE for the final block (critical tail)
                mc_eng = nc.vector
                mask_j = blk.tile([NL, MB], F32, tag="mask")
                mc_eng.tensor_scalar(
                    mask_j[:, :mb], acc[:, m0:m0 + mb], mnp[:, j:j + 1], None,
                    op0=OP.not_equal,
                )
                cand_j = blk.tile([NL, MB], F32, tag="cand")
                mc_eng.scalar_tensor_tensor(
                    cand_j[:, :mb],
                    mask_j[:, :mb],
                    BIG,
                    iota_t[:, m0:m0 + mb],
                    op0=OP.mult,
                    op1=OP.add,
                )
                nc.vector.tensor_reduce(
                    amins[:, j:j + 1], cand_j[:, :mb], axis=AX.X, op=OP.min
                )

            pending = None
            for j, (m0, mb) in enumerate(BLOCKS):
                d_j = dbuf.tile([NL, MB, P], F32, tag="d")
                for q in range(NQ):
                    s_t = ps.tile([NL, QUAD, MB], F32, tag="s")
                    for dp in range(QUAD):
                        p = QUAD * q + dp
                        t, e = p // TRIO, p % TRIO
                        g = gtTa if t < 2 else gtTb
                        ts_ = t if t < 2 else t - 2
                        nc.tensor.matmul(
                            s_t[:, dp, :mb],
                            lhsT=predT[32 * e:32 * e + 12, t, :],
                            rhs=g[32 * e:32 * e + 12, ts_, m0:m0 + mb],
                            start=True,
                            stop=True,
                        )
                    nc.scalar.activation(
                        d_j[:, :mb, QUAD * q:QUAD * (q + 1)].transpose([0, 2, 1]),
                        s_t[:, :, :mb],
                        FSQRT,
                        bias=bias_t[:],
                        scale=SCALE,
                    )
                accj = acc[:, m0:m0 + mb]
                if mb > 128:
                    # GPSIMD folds m[0:h] down to 5 p-planes while DVE folds
                    # + reduces m[h:mb]; by the time DVE reaches the pool's
                    # range both fold levels are done, so the in-order DVE
                    # queue never stalls. h sized to the measured ~2x
                    # GPSIMD/DVE rate ratio.
                    h = (mb * 33) // 64
                    nc.gpsimd.tensor_tensor(
                        d_j[:, 0:h, 0:P // 2],
                        d_j[:, 0:h, 0:P // 2],
                        d_j[:, 0:h, P // 2:P],
                        op=OP.add,
                    )
                    dve_fold = nc.vector.tensor_tensor(
                        d_j[:, h:mb, 0:P // 2],
                        d_j[:, h:mb, 0:P // 2],
                        d_j[:, h:mb, P // 2:P],
                        op=OP.add,
                    )
                    r_own = nc.vector.tensor_reduce(
                        acc[:, m0 + h:m0 + mb],
                        d_j[:, h:mb, 0:P // 2],
                        axis=AX.X,
                        op=OP.add,
                    )
                    r_pool = nc.vector.tensor_reduce(
                        acc[:, m0:m0 + h],
                        d_j[:, 0:h, 0:P // 2],
                        axis=AX.X,
                        op=OP.add,
                    )
                    # keep DVE busy on its own range while the pool fold runs
                    add_dep_helper(
                        r_pool.ins, r_own.ins, sync=False,
                        reason="reduce own range before pool range",
                    )
                else:
                    nc.vector.tensor_reduce(
                        accj, d_j[:, :mb, :], axis=AX.X, op=OP.add
                    )
                nc.vector.tensor_reduce(
                    mnp[:, j:j + 1], accj, axis=AX.X, op=OP.min
                )
                if pending is not None:
                    emit_chain(*pending, on_pool=True)
                pending = (j, m0, mb)
            emit_chain(*pending, on_pool=False)

            # ---- combine blocks ----
            mn = sb.tile([NL, 1], F32, tag="mn")
            nc.vector.tensor_reduce(mn[:], mnp[:], axis=AX.X, op=OP.min)
            bmask = sb.tile([NL, NB], F32, tag="bmask")
            nc.vector.tensor_scalar(
                bmask[:], mnp[:], mn[:], None, op0=OP.not_equal
            )
            cand2 = sb.tile([NL, NB], F32, tag="cand2")
            nc.vector.scalar_tensor_tensor(
                cand2[:], bmask[:], BIG, amins[:], op0=OP.mult, op1=OP.add
            )
            amin_f = sb.tile([NL, 1], F32, tag="aminf")
            nc.vector.tensor_reduce(amin_f[:], cand2[:], axis=AX.X, op=OP.min)
            idx_i = sb.tile([NL, 1], I32, tag="idx")
            nc.vector.tensor_copy(idx_i[:], amin_f[:])

            # ---- gather matched gt rows straight from DRAM ----
            matched_sb = sb.tile([NL, PC], F32, tag="matched")
            nc.gpsimd.indirect_dma_start(
                out=matched_sb[:],
                out_offset=None,
                in_=gt_d.ap().rearrange("m p c -> m (p c)"),
                in_offset=bass.IndirectOffsetOnAxis(ap=idx_i[:, :1], axis=0),
            )

            # ---- confidence = (mn <= thresh) * exp(-mn) ----
            # exp via DVE Horner polynomial: avoids swapping the ACT table
            # set (sqrt and exp live in different sets; a swap costs ~2.2us)
            tpoly = sb.tile([NL, 1], F32, tag="tpoly")
            upoly = sb.tile([NL, 1], F32, tag="upoly")
            nc.vector.tensor_scalar(
                upoly[:], mn[:], EXP_COEF[7], None, op0=OP.mult
            )
            nc.vector.tensor_scalar(
                tpoly[:], upoly[:], EXP_COEF[6], None, op0=OP.add
            )
            for k in range(5, -1, -1):
                nc.vector.tensor_tensor(upoly[:], tpoly[:], mn[:], op=OP.mult)
                nc.vector.tensor_scalar(
                    tpoly[:], upoly[:], EXP_COEF[k], None, op0=OP.add
                )
            cmask = sb.tile([NL, 1], F32, tag="cmask")
            nc.vector.tensor_scalar(
                cmask[:], mn[:], DIST_THRESHOLD, None, op0=OP.is_le
            )
            conf_sb = sb.tile([NL, 1], F32, tag="conf")
            nc.vector.tensor_tensor(conf_sb[:], tpoly[:], cmask[:], op=OP.mult)

            # ---- outputs ----
            nc.sync.dma_start(
                matched_d.ap().rearrange("n p c -> n (p c)"), matched_sb[:]
            )
            nc.sync.dma_start(conf_d.ap(), conf_sb[:])
            nc.sync.dma_start(idx_d.ap(), idx_i[:])

    nc.compile()
    return nc


def _get_nc():
    if "nc" not in _CACHE:
        _CACHE["nc"] = _build()
    return _CACHE["nc"]


def _run(in_maps, trace=False):
    nc = _get_nc()
    return run_bass_kernel_spmd(
        nc, in_maps, list(range(N_CORES)), trace=trace
    )


def make_in_maps(pred_points, gt_points):
    pred = np.ascontiguousarray(pred_points, dtype=np.float32)
    gt = np.ascontiguousarray(gt_points, dtype=np.float32)
    assert pred.shape == (N, P, C), pred.shape
    assert gt.shape == (M, P, C), gt.shape

    def split16(x):
        hi = x.astype(np.float16)
        lo = (x - hi.astype(np.float32)).astype(np.float16)
        return hi, lo

    gx, gy = gt[:, :, 0].T, gt[:, :, 1].T            # (P, M)
    B = np.zeros((P_PAD, 4, M), dtype=np.float32)
    B[:P, 0] = -2.0 * gx
    B[:P, 1] = -2.0 * gy
    B[:P, 2] = 1.0
    B[:P, 3] = gx * gx + gy * gy
    b_hi, b_lo = split16(B)
    gt_aug = np.ascontiguousarray(
        np.concatenate([b_hi, b_lo, b_hi], axis=1).reshape(12 * P_PAD, M)
    )

    maps = []
    for c in range(N_CORES):
        pr = pred[c * NL:(c + 1) * NL]
        px, py = pr[:, :, 0].T, pr[:, :, 1].T        # (P, NL)
        A = np.zeros((P_PAD, 4, NL), dtype=np.float32)
        A[:P, 0] = px
        A[:P, 1] = py
        A[:P, 2] = px * px + py * py
        A[:P, 3] = 1.0
        a_hi, a_lo = split16(A)
        pred_aug = np.ascontiguousarray(
            np.concatenate([a_hi, a_hi, a_lo], axis=1).reshape(12 * P_PAD, NL)
        )
        maps.append(
            {
                "pred_aug": pred_aug,
                "gt_aug": gt_aug,
                "gt": gt,
            }
        )
    return maps


def assemble(results):
    matched = np.concatenate([r["matched"] for r in results], axis=0)
    conf = np.concatenate([r["conf"] for r in results], axis=0)
    idx = np.concatenate([r["idx"][:, 0] for r in results], axis=0)
    return (
        matched.astype(np.float32),
        conf.astype(np.float32),
        idx.astype(np.int32),
    )


def kernel(pred_points, gt_points):
    res = _run(make_in_maps(pred_points, gt_points))
    return assemble(res.results)


# revision 52
# speedup vs baseline: 1.0203x; 1.0203x over previous
"""PointMatcher kernel for Trainium2 (8 NeuronCores, SPMD).

Sharding: pred_points (N=1024) split across 8 cores (128 rows = SBUF
partitions per core); gt_points replicated.

Per p-coordinate (P=20) the squared point-pair distance is a rank-4
bilinear form:
    s_p[n,m] = |p_n|^2 + |g_m|^2 - 2 p_n.g_m
             = [px, py, pn2, 1] . [-2gx, -2gy, 1, gn2]     (K=4)
The augmented operands are built host-side (cheap O(N*P) layout work) and
DMA'd in with each p-group at a 32-aligned partition base (PE quadrant
constraint). PE computes s via K=4 matmuls (1 col/cycle regardless of K),
ACT does sqrt(s/400 + eps) straight out of PSUM (eps guards fp32
cancellation-driven negatives; /400 folds the mean over P=20), DVE reduces
over P and does min/argmin per m-block, GPSIMD indirect-DMA gathers the
matched gt rows.
"""

import numpy as np

import concourse.bass as bass
import concourse.tile as tile
from concourse import bacc, mybir
from concourse.bass_utils import run_bass_kernel_spmd
from concourse.tile import add_dep_helper

N_CORES = 8
N, P, C = 1024, 20, 2
M = 2048
NL = N // N_CORES            # 128 pred rows per core == SBUF partitions
PC = P * C                   # 40
MB = 512                     # m-block (one PSUM bank of f32)
NJ = M // MB                 # 4 m-blocks
QUAD = 4                     # p's per PSUM tile / ACT instruction
NQ = P // QUAD               # 5
TRIO = 3                     # p-groups per partition layer (K base in {0,32,64})
SLOTS = 7                    # ceil(P / TRIO) free slots
P_PAD = TRIO * SLOTS         # 21 p-groups after zero padding
F32 = mybir.dt.float32
F16 = mybir.dt.float16
I32 = mybir.dt.int32
FSQRT = mybir.ActivationFunctionType.Sqrt
FEXP = mybir.ActivationFunctionType.Exp
OP = mybir.AluOpType
AX = mybir.AxisListType

SCALE = 1.0 / 400.0          # sqrt(s/400) == sqrt(s)/20 -> mean over P folded in
BIAS = 1e-5 * SCALE          # clamp fp16-split cancellation (s >= -1.8e-6 observed)
BIG = 4096.0                 # > any valid m index
DIST_THRESHOLD = 2.0
# m-blocks: small edge blocks shorten the DVE lead-in and tail
BLOCKS = [(0, 128), (128, 256), (384, 512), (896, 512), (1408, 512), (1920, 128)]
# exp(-x) on [0, 2.2], degree-7 (2.3e-6 max rel err in fp32); past the
# threshold the mask zeroes conf, so accuracy there is irrelevant
EXP_COEF = [
    0.999998982993151, -0.9999732918541867, 0.4998190768987987,
    -0.1661297557924529, 0.040819438656213146, -0.007559750384418043,
    0.0009681929279338809, -6.342769096779997e-05,
]

_CACHE: dict = {}


def _build():
    nc = bacc.Bacc("TRN2", target_bir_lowering=False, debug=False)

    pred_d = nc.dram_tensor("pred_aug", [12 * P_PAD, NL], F16, kind="ExternalInput")
    gta_d = nc.dram_tensor("gt_aug", [12 * P_PAD, M], F16, kind="ExternalInput")
    gt_d = nc.dram_tensor("gt", [M, P, C], F32, kind="ExternalInput")
    matched_d = nc.dram_tensor("matched", [NL, P, C], F32, kind="ExternalOutput")
    conf_d = nc.dram_tensor("conf", [NL, 1], F32, kind="ExternalOutput")
    idx_d = nc.dram_tensor("idx", [NL, 1], I32, kind="ExternalOutput")

    with tile.TileContext(nc) as tc:
        with (
            tc.tile_pool(name="sb", bufs=1) as sb,
            tc.tile_pool(name="dbuf", bufs=3) as dbuf,
            tc.tile_pool(name="blk", bufs=2) as blk,
            tc.tile_pool(name="ps", bufs=2, space=bass.MemorySpace.PSUM) as ps,
        ):
            # ---- load operands: p-group (3t+e) lives at partitions
            # [32e, 32e+12), free slot t. ----
            # One K=12 matmul per (p, block) computes the exact-enough product:
            # s = [A_hi, A_hi, A_lo] . [B_hi, B_lo, B_hi]  (A_lo.B_lo ~2^-24 dropped)
            # gt tensor split into head (slots 0-1) / tail (slots 2-6) tiles
            # so the first matmuls start after a small fraction of the DMA.
            predT = sb.tile([128, SLOTS, NL], F16, tag="predT")
            gtTa = sb.tile([128, 2, M], F16, tag="gtTa")
            gtTb = sb.tile([128, SLOTS - 2, M], F16, tag="gtTb")
            pred_v = pred_d.ap().rearrange(
                "(t e c) n -> e c t n", t=SLOTS, e=TRIO, c=12
            )
            gta_v = gta_d.ap().rearrange(
                "(t e c) m -> e c t m", t=SLOTS, e=TRIO, c=12
            )
            for e in range(TRIO):
                nc.sync.dma_start(
                    gtTa[32 * e:32 * e + 12, :, :], gta_v[e][:, 0:2, :]
                )
                nc.sync.dma_start(
                    predT[32 * e:32 * e + 12, :, :], pred_v[e]
                )
            for e in range(TRIO):
                nc.sync.dma_start(
                    gtTb[32 * e:32 * e + 12, :, :], gta_v[e][:, 2:, :]
                )

            iota_t = sb.tile([NL, M], F32, tag="iota")
            nc.gpsimd.iota(
                iota_t[:],
                [[1, M]],
                channel_multiplier=0,
                allow_small_or_imprecise_dtypes=True,
            )
            bias_t = sb.tile([NL, 1], F32, tag="bias")
            nc.gpsimd.memset(bias_t[:], BIAS)
            # data-independent dummy sqrt: pulls the ACT table load to t=0
            scratch = sb.tile([NL, 1], F32, tag="scr")
            nc.scalar.activation(scratch[:], bias_t[:], FSQRT)

            # ---- main loop: matmul -> sqrt -> reduce over P -> block min/argmin ----
            NB = len(BLOCKS)
            acc = sb.tile([NL, M], F32, tag="acc")
            mnp = sb.tile([NL, NB], F32, tag="mnp")
            amins = sb.tile([NL, NB], F32, tag="amins")

            def emit_chain(j, m0, mb, on_pool):
                # min/argmin candidate chain for block j; mask/cand on the
                # pool mid-loop (its inputs are ready well before the pool
                # reaches them), on DVE for the final block (critical tail)
                # NOTE: tensor_scalar with an AP scalar is not a legal Pool
                # opcode (walrus codegen rejects it) — keep the chain on DVE
                del on_pool
                mc_eng = nc.vector
                mask_j = blk.tile([NL, MB], F32, tag="mask")
                mc_eng.tensor_scalar(
                    mask_j[:, :mb], acc[:, m0:m0 + mb], mnp[:, j:j + 1], None,
                    op0=OP.not_equal,
                )
                cand_j = blk.tile([NL, MB], F32, tag="cand")
                mc_eng.scalar_tensor_tensor(
                    cand_j[:, :mb],
                    mask_j[:, :mb],
                    BIG,
                    iota_t[:, m0:m0 + mb],
                    op0=OP.mult,
                    op1=OP.add,
                )
                nc.vector.tensor_reduce(
                    amins[:, j:j + 1], cand_j[:, :mb], axis=AX.X, op=OP.min
                )

            pending = None
            for j, (m0, mb) in enumerate(BLOCKS):
                d_j = dbuf.tile([NL, MB, P], F32, tag="d")
                for q in range(NQ):
                    s_t = ps.tile([NL, QUAD, MB], F32, tag="s")
                    for dp in range(QUAD):
                        p = QUAD * q + dp
                        t, e = p // TRIO, p % TRIO
                        g = gtTa if t < 2 else gtTb
                        ts_ = t if t < 2 else t - 2
                        nc.tensor.matmul(
                            s_t[:, dp, :mb],
                            lhsT=predT[32 * e:32 * e + 12, t, :],
                            rhs=g[32 * e:32 * e + 12, ts_, m0:m0 + mb],
                            start=True,
                            stop=True,
                        )
                    nc.scalar.activation(
                        d_j[:, :mb, QUAD * q:QUAD * (q + 1)].transpose([0, 2, 1]),
                        s_t[:, :, :mb],
                        FSQRT,
                        bias=bias_t[:],
                        scale=SCALE,
                    )
                accj = acc[:, m0:m0 + mb]
                if mb > 128:
                    # GPSIMD folds m[0:h] in place while DVE folds + reduces
                    # m[h:mb]; by the time DVE reaches the pool's range the
                    # fold is done, so the in-order DVE queue never stalls.
                    # h sized to the measured ~2x GPSIMD/DVE rate ratio.
                    h = (mb * 33) // 64
                    nc.gpsimd.tensor_tensor(
                        d_j[:, 0:h, 0:P // 2],
                        d_j[:, 0:h, 0:P // 2],
                        d_j[:, 0:h, P // 2:P],
                        op=OP.add,
                    )
                    dve_fold = nc.vector.tensor_tensor(
                        d_j[:, h:mb, 0:P // 2],
                        d_j[:, h:mb, 0:P // 2],
                        d_j[:, h:mb, P // 2:P],
                        op=OP.add,
                    )
                    r_own = nc.vector.tensor_reduce(
                        acc[:, m0 + h:m0 + mb],
                        d_j[:, h:mb, 0:P // 2],
                        axis=AX.X,
                        op=OP.add,
                    )
                    r_pool = nc.vector.tensor_reduce(
                        acc[:, m0:m0 + h],
                        d_j[:, 0:h, 0:P // 2],
                        axis=AX.X,
                        op=OP.add,
                    )
                    # keep DVE busy on its own range while the pool fold runs
                    add_dep_helper(
                        r_pool.ins, r_own.ins, sync=False,
                        reason="reduce own range before pool range",
                    )
                elif j == len(BLOCKS) - 1:
                    # final block: pool folds the whole range while DVE is
                    # still draining earlier blocks; halves the tail reduce
                    nc.gpsimd.tensor_tensor(
                        d_j[:, :mb, 0:P // 2],
                        d_j[:, :mb, 0:P // 2],
                        d_j[:, :mb, P // 2:P],
                        op=OP.add,
                    )
                    nc.vector.tensor_reduce(
                        accj, d_j[:, :mb, 0:P // 2], axis=AX.X, op=OP.add
                    )
                else:
                    nc.vector.tensor_reduce(
                        accj, d_j[:, :mb, :], axis=AX.X, op=OP.add
                    )
                nc.vector.tensor_reduce(
                    mnp[:, j:j + 1], accj, axis=AX.X, op=OP.min
                )
                if pending is not None:
                    # only the chain landing in the DVE tail moves to the
                    # (by then idle) pool; earlier chains stay on DVE to
                    # avoid cross-engine stalls in the packed mid-stream
                    emit_chain(*pending, on_pool=(j == len(BLOCKS) - 1))
                pending = (j, m0, mb)
            emit_chain(*pending, on_pool=False)

            # ---- combine blocks ----
            mn = sb.tile([NL, 1], F32, tag="mn")
            nc.vector.tensor_reduce(mn[:], mnp[:], axis=AX.X, op=OP.min)
            bmask = sb.tile([NL, NB], F32, tag="bmask")
            nc.vector.tensor_scalar(
                bmask[:], mnp[:], mn[:], None, op0=OP.not_equal
            )
            cand2 = sb.tile([NL, NB], F32, tag="cand2")
            nc.vector.scalar_tensor_tensor(
                cand2[:], bmask[:], BIG, amins[:], op0=OP.mult, op1=OP.add
            )
            amin_f = sb.tile([NL, 1], F32, tag="aminf")
            nc.vector.tensor_reduce(amin_f[:], cand2[:], axis=AX.X, op=OP.min)
            idx_i = sb.tile([NL, 1], I32, tag="idx")
            nc.vector.tensor_copy(idx_i[:], amin_f[:])

            # ---- gather matched gt rows straight from DRAM ----
            matched_sb = sb.tile([NL, PC], F32, tag="matched")
            nc.gpsimd.indirect_dma_start(
                out=matched_sb[:],
                out_offset=None,
                in_=gt_d.ap().rearrange("m p c -> m (p c)"),
                in_offset=bass.IndirectOffsetOnAxis(ap=idx_i[:, :1], axis=0),
            )

            # ---- confidence = (mn <= thresh) * exp(-mn) ----
            # exp via DVE Horner polynomial: avoids swapping the ACT table
            # set (sqrt and exp live in different sets; a swap costs ~2.2us)
            tpoly = sb.tile([NL, 1], F32, tag="tpoly")
            upoly = sb.tile([NL, 1], F32, tag="upoly")
            nc.vector.tensor_scalar(
                upoly[:], mn[:], EXP_COEF[7], None, op0=OP.mult
            )
            nc.vector.tensor_scalar(
                tpoly[:], upoly[:], EXP_COEF[6], None, op0=OP.add
            )
            for k in range(5, -1, -1):
                nc.vector.tensor_tensor(upoly[:], tpoly[:], mn[:], op=OP.mult)
                nc.vector.tensor_scalar(
                    tpoly[:], upoly[:], EXP_COEF[k], None, op0=OP.add
                )
            cmask = sb.tile([NL, 1], F32, tag="cmask")
            nc.vector.tensor_scalar(
                cmask[:], mn[:], DIST_THRESHOLD, None, op0=OP.is_le
            )
            conf_sb = sb.tile([NL, 1], F32, tag="conf")
            nc.vector.tensor_tensor(conf_sb[:], tpoly[:], cmask[:], op=OP.mult)

            # ---- outputs ----
            nc.sync.dma_start(
                matched_d.ap().rearrange("n p c -> n (p c)"), matched_sb[:]
            )
            nc.sync.dma_start(conf_d.ap(), conf_sb[:])
            nc.sync.dma_start(idx_d.ap(), idx_i[:])

    nc.compile()
    return nc


def _get_nc():
    if "nc" not in _CACHE:
        _CACHE["nc"] = _build()
    return _CACHE["nc"]


def _run(in_maps, trace=False):
    nc = _get_nc()
    return run_bass_kernel_spmd(
        nc, in_maps, list(range(N_CORES)), trace=trace
    )


def make_in_maps(pred_points, gt_points):
    pred = np.ascontiguousarray(pred_points, dtype=np.float32)
    gt = np.ascontiguousarray(gt_points, dtype=np.float32)
    assert pred.shape == (N, P, C), pred.shape
    assert gt.shape == (M, P, C), gt.shape

    def split16(x):
        hi = x.astype(np.float16)
        lo = (x - hi.astype(np.float32)).astype(np.float16)
        return hi, lo

    gx, gy = gt[:, :, 0].T, gt[:, :, 1].T            # (P, M)
    B = np.zeros((P_PAD, 4, M), dtype=np.float32)
    B[:P, 0] = -2.0 * gx
    B[:P, 1] = -2.0 * gy
    B[:P, 2] = 1.0
    B[:P, 3] = gx * gx + gy * gy
    b_hi, b_lo = split16(B)
    gt_aug = np.ascontiguousarray(
        np.concatenate([b_hi, b_lo, b_hi], axis=1).reshape(12 * P_PAD, M)
    )

    maps = []
    for c in range(N_CORES):
        pr = pred[c * NL:(c + 1) * NL]
        px, py = pr[:, :, 0].T, pr[:, :, 1].T        # (P, NL)
        A = np.zeros((P_PAD, 4, NL), dtype=np.float32)
        A[:P, 0] = px
        A[:P, 1] = py
        A[:P, 2] = px * px + py * py
        A[:P, 3] = 1.0
        a_hi, a_lo = split16(A)
        pred_aug = np.ascontiguousarray(
            np.concatenate([a_hi, a_hi, a_lo], axis=1).reshape(12 * P_PAD, NL)
        )
        maps.append(
            {
                "pred_aug": pred_aug,
                "gt_aug": gt_aug,
                "gt": gt,
            }
        )
    return maps


def assemble(results):
    matched = np.concatenate([r["matched"] for r in results], axis=0)
    conf = np.concatenate([r["conf"] for r in results], axis=0)
    idx = np.concatenate([r["idx"][:, 0] for r in results], axis=0)
    return (
        matched.astype(np.float32),
        conf.astype(np.float32),
        idx.astype(np.int32),
    )


def kernel(pred_points, gt_points):
    res = _run(make_in_maps(pred_points, gt_points))
    return assemble(res.results)
